# revision 18
# baseline (speedup 1.0000x reference)
"""MoE block (B=16, C=192, H=W=32, E=8, top-2, 3x3 same-conv experts) on 8 trn2 cores.

Strategy:
  - Router (tiny: pool -> 192x8 matmul -> softmax -> top2) computed on host in numpy.
  - Conv is linear in weights, so the top-2 expert combine folds into ONE conv
    per sample with host-combined weights:
        out[b] = conv(x[b], sum_k w_bk * W_ek) + sum_k w_bk * b_ek
    Device work: 16 convs total -> 2 per core (data-parallel over batch).
  - Each conv = 9 shifted bf16 matmuls (taps) accumulating in fp32 PSUM;
    contract =
    input channels (192 = 128 + 64), M = output channels (192 = 128 + 64),
    N = 512 pixels (half image).
  - PE-array packing: column tiling is rejected by walrus on TRN2, so only
    row tiling is used. Each K=64 leftover-channel tap runs as a row PAIR
    (tile_position rows 0/64) covering BOTH pixel blocks concurrently,
    writing two different PSUM banks. Partitions 64..127 of the TB x-tile and
    of the K64 weight tile hold duplicates of partitions 0..63.
    144 naive matmuls -> 108 PE slots.
"""

import numpy as np

B, C, H, W = 16, 192, 32, 32
E, TOPK = 8, 2
NCORES = 8
S = B // NCORES          # samples per core
PW = W + 2               # padded width 34
PP = PW * PW             # padded pixels 1156
HWP = H * W              # 1024
PBS = 512                # pixels per block
ROWS_PB = 16             # output rows per block
XROWS = [(0, 18), (15, 34)]   # padded-row range each pixel block needs
TAPS = [(t // 3, t % 3) for t in range(9)]
N_WARMUP = 12

_cache = {}


def _build_module():
    import concourse.tile as tile
    from concourse import bacc, mybir

    f32 = mybir.dt.float32
    f32r = mybir.dt.bfloat16  # compute dtype (variable name kept from the f32r variant)

    nc = bacc.Bacc("TRN2", target_bir_lowering=False, debug=False, num_devices=NCORES)
    xp_d = nc.dram_tensor("xp", [S, C, PP], f32r, kind="ExternalInput")
    wa_d = nc.dram_tensor("wa", [S, 128, 9 * C], f32r, kind="ExternalInput")
    wbb_d = nc.dram_tensor("wbb", [S, 64, 9 * C], f32r, kind="ExternalInput")
    bias_d = nc.dram_tensor("bias", [128, 4], f32, kind="ExternalInput")
    out_d = nc.dram_tensor("out", [S, C, HWP], f32, kind="ExternalOutput")

    with tile.TileContext(nc) as tc:
        with (
            tc.tile_pool(name="xin", bufs=1) as xin,
            tc.tile_pool(name="win", bufs=1) as win,
            tc.tile_pool(name="cst", bufs=1) as cst,
            tc.tile_pool(name="ps", bufs=3, space="PSUM") as ps,
            tc.tile_pool(name="pw", bufs=1, space="PSUM") as pw,
            tc.tile_pool(name="oev", bufs=4) as oev,
        ):
            # --- PE warmup: tiny matmuls on zeros keep the clock ramped while
            # input DMAs stream in.
            scr = cst.tile([128, 512], mybir.dt.bfloat16, name="scr", tag="scr")
            nc.vector.memset(scr[:], 0.0)
            ps_scr = pw.tile([128, 512], f32, name="ps_scr", tag="ps_scr")
            for i in range(N_WARMUP):
                nc.tensor.matmul(ps_scr[:], scr[:, 0:128], scr[:], start=True,
                                 stop=True, skip_group_check=True)

            bias_t = cst.tile([128, 4], f32, name="bias_t", tag="bias_t")

            Ta = {}   # (s, pb) -> [128, rows*34] ch0-127 chunk
            TB = {}   # s -> [128, 1156]: ch128-191, duplicated on both halves
            WaC = {}  # (s, c) -> weight chunks for taps 0-2 / 3-8
            WBB = {}  # s -> [128, 9*192] K64 weights, duplicated halves

            def emit_input_dmas(s):
                ta0 = xin.tile([128, 18 * PW], f32r, name=f"Ta{s}_0", tag=f"Ta{s}_0")
                nc.sync.dma_start(ta0[:], xp_d[s, 0:128, 0 : 18 * PW])
                Ta[(s, 0)] = ta0
                wac0 = win.tile([128, 3 * C], f32r, name=f"WaC{s}_0", tag=f"WaC{s}_0")
                nc.sync.dma_start(wac0[:], wa_d[s, :, 0 : 3 * C])
                WaC[(s, 0)] = wac0
                wac1 = win.tile([128, 6 * C], f32r, name=f"WaC{s}_1", tag=f"WaC{s}_1")
                nc.sync.dma_start(wac1[:], wa_d[s, :, 3 * C : 9 * C])
                WaC[(s, 1)] = wac1

                ta1 = xin.tile([128, 19 * PW], f32r, name=f"Ta{s}_1", tag=f"Ta{s}_1")
                nc.sync.dma_start(ta1[:], xp_d[s, 0:128, 15 * PW : 34 * PW])
                Ta[(s, 1)] = ta1

                # Lower half serves pixel-block-0 windows (padded rows 0..17),
                # upper half serves pixel-block-1 windows (rows 15..33) -- so
                # each half only needs its row range; no duplicate bytes.
                tb = xin.tile([128, PP], f32r, name=f"TB_{s}", tag=f"TB_{s}")
                nc.sync.dma_start(tb[0:64, 0 : 18 * PW], xp_d[s, 128:192, 0 : 18 * PW])
                nc.sync.dma_start(tb[64:128, 15 * PW : PP], xp_d[s, 128:192, 15 * PW : PP])
                TB[s] = tb

                wbb = win.tile([128, 9 * C], f32r, name=f"WBB{s}", tag=f"WBB{s}")
                nc.sync.dma_start(wbb[0:64, :], wbb_d[s])
                nc.sync.dma_start(wbb[64:128, :], wbb[0:64, :])
                WBB[s] = wbb
                if s == 0:
                    nc.sync.dma_start(bias_t[:], bias_d[:])

            def wa_tap(s, t):
                """lhsT slices for tap t: (K128 x M192 full)"""
                if t < 3:
                    return WaC[(s, 0)][:, t * C : (t + 1) * C]
                return WaC[(s, 1)][:, (t - 3) * C : (t - 2) * C]

            def ta_rhs(s, pb, t):
                dy, dx = TAPS[t]
                v = Ta[(s, pb)][:].rearrange("p (r c) -> p r c", c=PW)
                y = ROWS_PB * pb - XROWS[pb][0] + dy
                return v[:, y : y + ROWS_PB, dx : dx + W]

            def tb_rhs(s, half, pb, t):
                """K64 moving AP from the duplicated TB tile: partition half
                `half` (0 -> rows 0..63, 1 -> 64..127), tap-t window of pb."""
                dy, dx = TAPS[t]
                v = TB[s][:].rearrange("p (r c) -> p r c", c=PW)
                y = ROWS_PB * pb + dy
                return v[64 * half : 64 * half + 64, y : y + ROWS_PB, dx : dx + W]

            def emit_A(s, pb, psA_pb):
                for t in range(9):
                    nc.tensor.matmul(psA_pb[:], wa_tap(s, t)[:, 0:128],
                                     ta_rhs(s, pb, t), start=(t == 0), stop=False)

            def emit_B(s, psA):
                # tap t covers pixel blocks 0 (array rows 0..63) and 1
                # (rows 64..127) concurrently, into two different PSUM banks.
                for t in range(9):
                    nc.tensor.matmul(psA[0][:], WBB[s][0:64, t * C : t * C + 128],
                                     tb_rhs(s, 0, 0, t),
                                     start=False, stop=(t == 8))
                    nc.tensor.matmul(psA[1][:], WBB[s][64:128, t * C : t * C + 128],
                                     tb_rhs(s, 1, 1, t),
                                     start=False, stop=(t == 8))

            def emit_C(s, pb, psC_pb):
                for t in range(9):
                    nc.tensor.matmul(psC_pb[:], wa_tap(s, t)[:, 128:192],
                                     ta_rhs(s, pb, t), start=(t == 0), stop=False)

            def emit_D(s, psC):
                # row-paired like B: pb0 on rows 0..63, pb1 on rows 64..127.
                for t in range(9):
                    nc.tensor.matmul(psC[0][:], WBB[s][0:64, t * C + 128 : t * C + 192],
                                     tb_rhs(s, 0, 0, t),
                                     start=False, stop=(t == 8))
                    nc.tensor.matmul(psC[1][:], WBB[s][64:128, t * C + 128 : t * C + 192],
                                     tb_rhs(s, 1, 1, t),
                                     start=False, stop=(t == 8))

            def evict_A(s, pb, psA_pb):
                # DVE: PSUM + per-channel bias -> SBUF, then DMA out.
                oA = oev.tile([128, PBS], f32, name=f"oA_{s}_{pb}", tag="oA")
                nc.vector.tensor_scalar_add(oA[:], psA_pb[:], bias_t[:, s : s + 1])
                eng = nc.scalar if s == 0 else nc.sync
                eng.dma_start(out_d[s, 0:128, pb * PBS : (pb + 1) * PBS], oA[:])

            def evict_C(s, psC):
                # ACT: two [64,512] banks -> one [64,1024] tile, one DMA out.
                oC = oev.tile([64, HWP], f32, name=f"oC_{s}", tag="oC")
                for pb in range(2):
                    nc.scalar.activation(
                        oC[:, pb * PBS : (pb + 1) * PBS], psC[pb][:],
                        mybir.ActivationFunctionType.Identity,
                        bias=bias_t[0:64, 2 + s : 3 + s], scale=1.0,
                    )
                eng = nc.gpsimd if s == 0 else nc.sync
                eng.dma_start(out_d[s, 128:192, :], oC[:])

            def emit_sample(s):
                psA = [
                    ps.tile([128, PBS], f32, name=f"psA_{s}_{pb}", tag="psA")
                    for pb in range(2)
                ]
                psC = [
                    ps.tile([64, PBS], f32, name=f"psC_{s}_{pb}", tag="psC")
                    for pb in range(2)
                ]
                emit_A(s, 0, psA[0])
                emit_A(s, 1, psA[1])
                emit_C(s, 0, psC[0])
                emit_C(s, 1, psC[1])
                emit_B(s, psA)
                evict_A(s, 0, psA[0])
                evict_A(s, 1, psA[1])
                emit_D(s, psC)
                evict_C(s, psC)

            emit_input_dmas(0)
            emit_input_dmas(1)
            emit_sample(0)
            emit_sample(1)

    nc.compile()
    return nc


def get_module():
    if "nc" not in _cache:
        _cache["nc"] = _build_module()
    return _cache["nc"]


def _route(x, gate_w, gate_b):
    """Replicates the reference router in numpy fp32. Returns combine [B,E]."""
    pooled = x.mean(axis=(2, 3), dtype=np.float32)
    logits = pooled @ gate_w + gate_b
    z = logits - logits.max(axis=-1, keepdims=True)
    ez = np.exp(z)
    w = ez / ez.sum(axis=-1, keepdims=True)
    topi = np.argsort(-w, axis=-1, kind="stable")[:, :TOPK]
    topw = np.take_along_axis(w, topi, axis=-1)
    topw = topw / (topw.sum(-1, keepdims=True) + 1e-10)
    combine = np.zeros((B, E), np.float32)
    np.put_along_axis(combine, topi, topw, axis=-1)
    return combine


def make_in_maps(x, gate_w, gate_b, expert_w, expert_b):
    x = np.ascontiguousarray(np.asarray(x, np.float32))
    gate_w = np.asarray(gate_w, np.float32)
    gate_b = np.asarray(gate_b, np.float32)
    expert_w = np.asarray(expert_w, np.float32)
    expert_b = np.asarray(expert_b, np.float32)

    combine = _route(x, gate_w, gate_b)                       # [B,E]
    Wc = np.einsum("be,eoikl->boikl", combine, expert_w)      # [B,C,C,3,3]
    bc = combine @ expert_b                                   # [B,C]

    # Padded input images: [B, C, 34*34]
    xp = np.zeros((B, C, PW, PW), np.float32)
    xp[:, :, 1 : H + 1, 1 : W + 1] = x
    xp = xp.reshape(B, C, PP)

    # lhsT layout: WT[b, t, i, o] = Wc[b, o, i, dy, dx]
    WT = Wc.transpose(0, 3, 4, 2, 1).reshape(B, 9, C, C)      # [B, 9, in, out]
    # wa[b, p, t*192+m] = WT[b,t,p,m] for p<128
    wa = np.ascontiguousarray(
        WT[:, :, 0:128, :].transpose(0, 2, 1, 3).reshape(B, 128, 9 * C)
    )
    # K64 weights with duplicated partition halves:
    # wbb[b, p, t*192+m] = WT[b, t, 128 + (p % 64), m]
    wbb = np.ascontiguousarray(
        WT[:, :, 128:192, :].transpose(0, 2, 1, 3).reshape(B, 64, 9 * C)
    )

    import ml_dtypes
    bf16 = ml_dtypes.bfloat16
    xp = xp.astype(bf16)
    wa = wa.astype(bf16)
    wbb = wbb.astype(bf16)

    in_maps = []
    for c in range(NCORES):
        b0 = S * c
        bias = np.zeros((128, 4), np.float32)
        for s in range(S):
            bias[:, s] = bc[b0 + s, 0:128]
            bias[0:64, 2 + s] = bc[b0 + s, 128:192]
        in_maps.append(
            {
                "xp": np.ascontiguousarray(xp[b0 : b0 + S]),
                "wa": np.ascontiguousarray(wa[b0 : b0 + S]),
                "wbb": np.ascontiguousarray(wbb[b0 : b0 + S]),
                "bias": bias,
            }
        )
    return in_maps


def kernel(x, gate_w, gate_b, expert_w, expert_b):
    from concourse.bass_utils import run_bass_kernel_spmd

    nc = get_module()
    in_maps = make_in_maps(x, gate_w, gate_b, expert_w, expert_b)
    res = run_bass_kernel_spmd(nc, in_maps, core_ids=list(range(NCORES)))
    out = np.stack([res.results[c]["out"] for c in range(NCORES)])  # [8,S,C,HWP]
    return out.reshape(B, C, H, W)


# revision 19
# speedup vs baseline: 1.0024x; 1.0024x over previous
"""MoE block (B=16, C=192, H=W=32, E=8, top-2, 3x3 same-conv experts) on 8 trn2 cores.

Strategy:
  - Router (tiny: pool -> 192x8 matmul -> softmax -> top2) computed on host in numpy.
  - Conv is linear in weights, so the top-2 expert combine folds into ONE conv
    per sample with host-combined weights:
        out[b] = conv(x[b], sum_k w_bk * W_ek) + sum_k w_bk * b_ek
    Device work: 16 convs total -> 2 per core (data-parallel over batch).
  - Each conv = 9 shifted bf16 matmuls (taps) accumulating in fp32 PSUM;
    contract =
    input channels (192 = 128 + 64), M = output channels (192 = 128 + 64),
    N = 512 pixels (half image).
  - PE-array packing: column tiling is rejected by walrus on TRN2, so only
    row tiling is used. Each K=64 leftover-channel tap runs as a row PAIR
    (tile_position rows 0/64) covering BOTH pixel blocks concurrently,
    writing two different PSUM banks. Partitions 64..127 of the TB x-tile and
    of the K64 weight tile hold duplicates of partitions 0..63.
    144 naive matmuls -> 108 PE slots.
"""

import numpy as np

B, C, H, W = 16, 192, 32, 32
E, TOPK = 8, 2
NCORES = 8
S = B // NCORES          # samples per core
PW = W + 2               # padded width 34
PP = PW * PW             # padded pixels 1156
HWP = H * W              # 1024
PBS = 512                # pixels per block
ROWS_PB = 16             # output rows per block
XROWS = [(0, 18), (15, 34)]   # padded-row range each pixel block needs
TAPS = [(t // 3, t % 3) for t in range(9)]
N_WARMUP = 12

_cache = {}


def _build_module():
    import concourse.tile as tile
    from concourse import bacc, mybir

    f32 = mybir.dt.float32
    f32r = mybir.dt.bfloat16  # compute dtype (variable name kept from the f32r variant)

    nc = bacc.Bacc("TRN2", target_bir_lowering=False, debug=False, num_devices=NCORES)
    xp_d = nc.dram_tensor("xp", [S, C, PP], f32r, kind="ExternalInput")
    wa_d = nc.dram_tensor("wa", [S, 128, 9 * C], f32r, kind="ExternalInput")
    wbb_d = nc.dram_tensor("wbb", [S, 64, 9 * C], f32r, kind="ExternalInput")
    bias_d = nc.dram_tensor("bias", [128, 4], f32, kind="ExternalInput")
    out_d = nc.dram_tensor("out", [S, C, HWP], f32, kind="ExternalOutput")

    with tile.TileContext(nc) as tc:
        with (
            tc.tile_pool(name="xin", bufs=1) as xin,
            tc.tile_pool(name="win", bufs=1) as win,
            tc.tile_pool(name="cst", bufs=1) as cst,
            tc.tile_pool(name="ps", bufs=3, space="PSUM") as ps,
            tc.tile_pool(name="pw", bufs=1, space="PSUM") as pw,
            tc.tile_pool(name="oev", bufs=4) as oev,
        ):
            # --- PE warmup: tiny matmuls on zeros keep the clock ramped while
            # input DMAs stream in.
            scr = cst.tile([128, 512], mybir.dt.bfloat16, name="scr", tag="scr")
            nc.vector.memset(scr[:], 0.0)
            ps_scr = pw.tile([128, 512], f32, name="ps_scr", tag="ps_scr")
            for i in range(N_WARMUP):
                nc.tensor.matmul(ps_scr[:], scr[:, 0:128], scr[:], start=True,
                                 stop=True, skip_group_check=True)

            bias_t = cst.tile([128, 4], f32, name="bias_t", tag="bias_t")

            Ta = {}   # (s, pb) -> [128, rows*34] ch0-127 chunk
            TB = {}   # s -> [128, 1156]: ch128-191, duplicated on both halves
            WaC = {}  # (s, c) -> weight chunks for taps 0-2 / 3-8
            WBB = {}  # s -> [128, 9*192] K64 weights, duplicated halves

            def emit_input_dmas(s):
                ta0 = xin.tile([128, 18 * PW], f32r, name=f"Ta{s}_0", tag=f"Ta{s}_0")
                nc.sync.dma_start(ta0[:], xp_d[s, 0:128, 0 : 18 * PW])
                Ta[(s, 0)] = ta0
                wac0 = win.tile([128, 3 * C], f32r, name=f"WaC{s}_0", tag=f"WaC{s}_0")
                nc.scalar.dma_start(wac0[:], wa_d[s, :, 0 : 3 * C])
                WaC[(s, 0)] = wac0
                wac1 = win.tile([128, 6 * C], f32r, name=f"WaC{s}_1", tag=f"WaC{s}_1")
                nc.scalar.dma_start(wac1[:], wa_d[s, :, 3 * C : 9 * C])
                WaC[(s, 1)] = wac1

                ta1 = xin.tile([128, 19 * PW], f32r, name=f"Ta{s}_1", tag=f"Ta{s}_1")
                nc.sync.dma_start(ta1[:], xp_d[s, 0:128, 15 * PW : 34 * PW])
                Ta[(s, 1)] = ta1

                # Lower half serves pixel-block-0 windows (padded rows 0..17),
                # upper half serves pixel-block-1 windows (rows 15..33) -- so
                # each half only needs its row range; no duplicate bytes.
                tb = xin.tile([128, PP], f32r, name=f"TB_{s}", tag=f"TB_{s}")
                nc.sync.dma_start(tb[0:64, 0 : 18 * PW], xp_d[s, 128:192, 0 : 18 * PW])
                nc.gpsimd.dma_start(tb[64:128, 15 * PW : PP], xp_d[s, 128:192, 15 * PW : PP])
                TB[s] = tb

                wbb = win.tile([128, 9 * C], f32r, name=f"WBB{s}", tag=f"WBB{s}")
                nc.gpsimd.dma_start(wbb[0:64, :], wbb_d[s])
                nc.gpsimd.dma_start(wbb[64:128, :], wbb[0:64, :])
                WBB[s] = wbb
                if s == 0:
                    nc.sync.dma_start(bias_t[:], bias_d[:])

            def wa_tap(s, t):
                """lhsT slices for tap t: (K128 x M192 full)"""
                if t < 3:
                    return WaC[(s, 0)][:, t * C : (t + 1) * C]
                return WaC[(s, 1)][:, (t - 3) * C : (t - 2) * C]

            def ta_rhs(s, pb, t):
                dy, dx = TAPS[t]
                v = Ta[(s, pb)][:].rearrange("p (r c) -> p r c", c=PW)
                y = ROWS_PB * pb - XROWS[pb][0] + dy
                return v[:, y : y + ROWS_PB, dx : dx + W]

            def tb_rhs(s, half, pb, t):
                """K64 moving AP from the duplicated TB tile: partition half
                `half` (0 -> rows 0..63, 1 -> 64..127), tap-t window of pb."""
                dy, dx = TAPS[t]
                v = TB[s][:].rearrange("p (r c) -> p r c", c=PW)
                y = ROWS_PB * pb + dy
                return v[64 * half : 64 * half + 64, y : y + ROWS_PB, dx : dx + W]

            def emit_A(s, pb, psA_pb):
                for t in range(9):
                    nc.tensor.matmul(psA_pb[:], wa_tap(s, t)[:, 0:128],
                                     ta_rhs(s, pb, t), start=(t == 0), stop=False)

            def emit_B(s, psA):
                # tap t covers pixel blocks 0 (array rows 0..63) and 1
                # (rows 64..127) concurrently, into two different PSUM banks.
                for t in range(9):
                    nc.tensor.matmul(psA[0][:], WBB[s][0:64, t * C : t * C + 128],
                                     tb_rhs(s, 0, 0, t),
                                     start=False, stop=(t == 8))
                    nc.tensor.matmul(psA[1][:], WBB[s][64:128, t * C : t * C + 128],
                                     tb_rhs(s, 1, 1, t),
                                     start=False, stop=(t == 8))

            def emit_C(s, pb, psC_pb):
                for t in range(9):
                    nc.tensor.matmul(psC_pb[:], wa_tap(s, t)[:, 128:192],
                                     ta_rhs(s, pb, t), start=(t == 0), stop=False)

            def emit_D(s, psC):
                # row-paired like B: pb0 on rows 0..63, pb1 on rows 64..127.
                for t in range(9):
                    nc.tensor.matmul(psC[0][:], WBB[s][0:64, t * C + 128 : t * C + 192],
                                     tb_rhs(s, 0, 0, t),
                                     start=False, stop=(t == 8))
                    nc.tensor.matmul(psC[1][:], WBB[s][64:128, t * C + 128 : t * C + 192],
                                     tb_rhs(s, 1, 1, t),
                                     start=False, stop=(t == 8))

            def evict_A(s, pb, psA_pb):
                # DVE: PSUM + per-channel bias -> SBUF, then DMA out.
                oA = oev.tile([128, PBS], f32, name=f"oA_{s}_{pb}", tag="oA")
                nc.vector.tensor_scalar_add(oA[:], psA_pb[:], bias_t[:, s : s + 1])
                eng = nc.scalar if s == 0 else nc.sync
                eng.dma_start(out_d[s, 0:128, pb * PBS : (pb + 1) * PBS], oA[:])

            def evict_C(s, psC):
                # ACT: two [64,512] banks -> one [64,1024] tile, one DMA out.
                oC = oev.tile([64, HWP], f32, name=f"oC_{s}", tag="oC")
                for pb in range(2):
                    nc.scalar.activation(
                        oC[:, pb * PBS : (pb + 1) * PBS], psC[pb][:],
                        mybir.ActivationFunctionType.Identity,
                        bias=bias_t[0:64, 2 + s : 3 + s], scale=1.0,
                    )
                eng = nc.gpsimd if s == 0 else nc.sync
                eng.dma_start(out_d[s, 128:192, :], oC[:])

            def emit_sample(s):
                psA = [
                    ps.tile([128, PBS], f32, name=f"psA_{s}_{pb}", tag="psA")
                    for pb in range(2)
                ]
                psC = [
                    ps.tile([64, PBS], f32, name=f"psC_{s}_{pb}", tag="psC")
                    for pb in range(2)
                ]
                emit_A(s, 0, psA[0])
                emit_A(s, 1, psA[1])
                emit_C(s, 0, psC[0])
                emit_C(s, 1, psC[1])
                emit_B(s, psA)
                evict_A(s, 0, psA[0])
                evict_A(s, 1, psA[1])
                emit_D(s, psC)
                evict_C(s, psC)

            emit_input_dmas(0)
            emit_input_dmas(1)
            emit_sample(0)
            emit_sample(1)

    nc.compile()
    return nc


def get_module():
    if "nc" not in _cache:
        _cache["nc"] = _build_module()
    return _cache["nc"]


def _route(x, gate_w, gate_b):
    """Replicates the reference router in numpy fp32. Returns combine [B,E]."""
    pooled = x.mean(axis=(2, 3), dtype=np.float32)
    logits = pooled @ gate_w + gate_b
    z = logits - logits.max(axis=-1, keepdims=True)
    ez = np.exp(z)
    w = ez / ez.sum(axis=-1, keepdims=True)
    topi = np.argsort(-w, axis=-1, kind="stable")[:, :TOPK]
    topw = np.take_along_axis(w, topi, axis=-1)
    topw = topw / (topw.sum(-1, keepdims=True) + 1e-10)
    combine = np.zeros((B, E), np.float32)
    np.put_along_axis(combine, topi, topw, axis=-1)
    return combine


def make_in_maps(x, gate_w, gate_b, expert_w, expert_b):
    x = np.ascontiguousarray(np.asarray(x, np.float32))
    gate_w = np.asarray(gate_w, np.float32)
    gate_b = np.asarray(gate_b, np.float32)
    expert_w = np.asarray(expert_w, np.float32)
    expert_b = np.asarray(expert_b, np.float32)

    combine = _route(x, gate_w, gate_b)                       # [B,E]
    Wc = np.einsum("be,eoikl->boikl", combine, expert_w)      # [B,C,C,3,3]
    bc = combine @ expert_b                                   # [B,C]

    # Padded input images: [B, C, 34*34]
    xp = np.zeros((B, C, PW, PW), np.float32)
    xp[:, :, 1 : H + 1, 1 : W + 1] = x
    xp = xp.reshape(B, C, PP)

    # lhsT layout: WT[b, t, i, o] = Wc[b, o, i, dy, dx]
    WT = Wc.transpose(0, 3, 4, 2, 1).reshape(B, 9, C, C)      # [B, 9, in, out]
    # wa[b, p, t*192+m] = WT[b,t,p,m] for p<128
    wa = np.ascontiguousarray(
        WT[:, :, 0:128, :].transpose(0, 2, 1, 3).reshape(B, 128, 9 * C)
    )
    # K64 weights with duplicated partition halves:
    # wbb[b, p, t*192+m] = WT[b, t, 128 + (p % 64), m]
    wbb = np.ascontiguousarray(
        WT[:, :, 128:192, :].transpose(0, 2, 1, 3).reshape(B, 64, 9 * C)
    )

    import ml_dtypes
    bf16 = ml_dtypes.bfloat16
    xp = xp.astype(bf16)
    wa = wa.astype(bf16)
    wbb = wbb.astype(bf16)

    in_maps = []
    for c in range(NCORES):
        b0 = S * c
        bias = np.zeros((128, 4), np.float32)
        for s in range(S):
            bias[:, s] = bc[b0 + s, 0:128]
            bias[0:64, 2 + s] = bc[b0 + s, 128:192]
        in_maps.append(
            {
                "xp": np.ascontiguousarray(xp[b0 : b0 + S]),
                "wa": np.ascontiguousarray(wa[b0 : b0 + S]),
                "wbb": np.ascontiguousarray(wbb[b0 : b0 + S]),
                "bias": bias,
            }
        )
    return in_maps


def kernel(x, gate_w, gate_b, expert_w, expert_b):
    from concourse.bass_utils import run_bass_kernel_spmd

    nc = get_module()
    in_maps = make_in_maps(x, gate_w, gate_b, expert_w, expert_b)
    res = run_bass_kernel_spmd(nc, in_maps, core_ids=list(range(NCORES)))
    out = np.stack([res.results[c]["out"] for c in range(NCORES)])  # [8,S,C,HWP]
    return out.reshape(B, C, H, W)
